# revision 93
# baseline (speedup 1.0000x reference)
"""MACE-style GNN message passing on 8 Trainium2 NeuronCores.

Only the l=0 (scalar) channel of the reference network reaches the output:
Y[:,0] == 1 and the readout consumes out[:, :, 0] alone, so the whole l>=1
spherical-harmonic pipeline is dead code.  What remains per edge is the
radial MLP (8->64->64->64->128), a per-sender-species channel scale, and a
scatter-add over receivers.  Node-side work collapses to per-species table
lookups (implemented as one-hot matmuls) plus three K x K matmuls.

Sharding: receivers are range-partitioned over the 8 cores (1000 nodes
each, padded to 8 tiles of 128).  Each core gets the edges targeting its
nodes, grouped by 128-node tile and padded to a uniform segment length so
all cores run one SPMD program.  Edges with r >= r_max (zero envelope) are
dropped on the host.

All matmuls keep operands on full 128-partition tiles at base partition 0
(tile_position (0,0)); sub-128 contractions are realized with zero-padded
block weights.  Partial-partition operands on compute-written tiles
misread on this hardware.
"""

import sys
import numpy as np

sys.path.insert(0, "/opt/trn_rl_repo")

R_MAX = 5.0
EPS = 1e-9
AVG = 16.0
N_NODES = 8000
Z = 10
K = 128
NB = 8
NCORES = 8
NPC = N_NODES // NCORES       # nodes per core
NT = 8                        # node tiles per core (128 nodes each)
NPAD = NT * 128               # padded nodes per core
ET_EDGES = 512                # edges per device tile (4 x 128 subtiles)

# fp16 constants (matmul weights; PE runs fp16 at 1 cycle/row vs 4 for fp32)
_CONSTH_SPECS = [
    ("i128", 128), ("iota", 128),
    ("w1a", 128), ("w1b", 128),
    ("w2", 128), ("w3", 128),
    ("w4eo", 256),
    ("wlin0", 128), ("wsym0", 128),
    ("wlin20", 128), ("sct", 128), ("ae", 1), ("wro", 1),
]
CONSTH_W = sum(w for _, w in _CONSTH_SPECS)


def _spec_cols(specs):
    cols, off = {}, 0
    for name, w in specs:
        cols[name] = (off, off + w)
        off += w
    return cols


TRACE = False
LAST_RESULTS = None

_prog_cache = {}


def _build_program(padded):
    """Build the SPMD Bass program.  `padded` = slots per node tile
    (multiples of 128, shared across cores); one 512-edge device tile
    may span node tiles — scatter targets are baked per subtile."""
    from concourse import bass, bacc, mybir
    from concourse.tile import TileContext

    f32 = mybir.dt.float32
    f16 = mybir.dt.float16
    f8e4 = mybir.dt.float8e4
    f8e5 = mybir.dt.float8e5
    i32 = mybir.dt.int32
    AF = mybir.ActivationFunctionType
    OP = mybir.AluOpType
    DR = mybir.MatmulPerfMode.DoubleRow

    nc = bacc.Bacc(None, target_bir_lowering=False)
    tot = sum(padded)
    NTT = -(-tot // 1024) * 2   # device tiles per core (even: pair loop)
    NS = NTT * 4           # 128-edge subtiles
    real_NS = tot // 128   # live subtiles; dead tail skipped in scatter
    # subtile -> node tile schedule
    sched = []
    for t, p in enumerate(padded):
        sched += [t] * (p // 128)
    sched += [NT - 1] * (NS - len(sched))
    first_S = {}
    last_S = {}
    for S in range(real_NS):
        first_S.setdefault(sched[S], S)
        last_S[sched[S]] = S
    NE4 = NTT * 4          # (row, tile, subtile) scalar lanes
    NE32 = NTT * 32        # bessel feature lanes

    ef_d = nc.dram_tensor("ef", [128, NE32], f16, kind="ExternalInput")
    rl_d = nc.dram_tensor("rl", [128, NE4], f16, kind="ExternalInput")
    ht_d = nc.dram_tensor("ht", [NTT, 128, 512], f16, kind="ExternalInput")
    ohn_d = nc.dram_tensor("ohn", [10, NPAD], f16, kind="ExternalInput")
    consth_d = nc.dram_tensor("consth", [128, CONSTH_W], f16, kind="ExternalInput")
    out_d = nc.dram_tensor("out", [1, NPAD], f32, kind="ExternalOutput")

    PSUM = bass.MemorySpace.PSUM

    from contextlib import ExitStack

    with TileContext(nc) as tc:
        with ExitStack() as stack:
            cp = stack.enter_context(tc.tile_pool(name="const", bufs=1))
            htp = stack.enter_context(tc.tile_pool(name="htp", bufs=3))
            tp1 = stack.enter_context(tc.tile_pool(name="tp1", bufs=4))
            tp2 = stack.enter_context(tc.tile_pool(name="tp2", bufs=4))
            tp3 = stack.enter_context(tc.tile_pool(name="tp3", bufs=4))
            sap = stack.enter_context(tc.tile_pool(name="sap", bufs=3))
            ohrp = stack.enter_context(tc.tile_pool(name="ohrp", bufs=3))
            nsb = stack.enter_context(tc.tile_pool(name="nsb", bufs=2))
            pmlp = stack.enter_context(tc.tile_pool(name="pmlp", bufs=3, space=PSUM))
            pbig = stack.enter_context(tc.tile_pool(name="pbig", bufs=2, space=PSUM))
            pmsg = stack.enter_context(tc.tile_pool(name="pmsg", bufs=2, space=PSUM))

            # ---- inputs: weights + edge features first (critical path)
            CTH = cp.tile([128, CONSTH_W], f16)
            nc.sync.dma_start(CTH[:], consth_d[:])
            EFALL = cp.tile([128, NE32], f16)
            nc.sync.dma_start(EFALL[:], ef_d[:])
            RL = cp.tile([128, NE4], f16)
            nc.sync.dma_start(RL[:], rl_d[:])
            OHN = cp.tile([128, NPAD], f16)
            nc.sync.dma_start(OHN[0:10, :], ohn_d[:])
            ch = _spec_cols(_CONSTH_SPECS)
            IOTA = CTH[:, ch["iota"][0]:ch["iota"][1]]
            I128 = CTH[:, ch["i128"][0]:ch["i128"][1]]
            W1A = CTH[:, ch["w1a"][0]:ch["w1a"][1]]
            W1B = CTH[:, ch["w1b"][0]:ch["w1b"][1]]
            W2 = CTH[:, ch["w2"][0]:ch["w2"][1]]
            W3 = CTH[:, ch["w3"][0]:ch["w3"][1]]
            W4EO = CTH[:, ch["w4eo"][0]:ch["w4eo"][1]]
            WLIN0 = CTH[:, ch["wlin0"][0]:ch["wlin0"][1]]
            WSYM0 = CTH[0:10, ch["wsym0"][0]:ch["wsym0"][1]]
            WLIN20 = CTH[:, ch["wlin20"][0]:ch["wlin20"][1]]
            SCT = CTH[0:10, ch["sct"][0]:ch["sct"][1]]
            AE = CTH[0:10, ch["ae"][0]:ch["ae"][1]]
            WRO = CTH[:, ch["wro"][0]:ch["wro"][1]]
            OUT = cp.tile([1, NPAD], f32)
            MS = cp.tile([128, NPAD], f16)   # all node-tile messages

            # rotating efT buffers (2 tiles each); rows 32:128 stay zero
            efTs = [cp.tile([128, 256], f16, name=f"efT{i}") for i in range(3)]
            for b_ in efTs:
                nc.gpsimd.memset(b_[:], 0.0)

            def epilogue(hb):
                # per-512-node block: mix messages, element coef, readout
                sl0, sl1 = hb * 512, hb * 512 + 512
                pf = pbig.tile([128, 512], f32, tag="pbig", name=f"pf{hb}")
                nc.tensor.matmul(pf[:], WLIN0, MS[:, sl0:sl1],
                                 start=True, stop=True)
                ohn_t = OHN[0:10, sl0:sl1]
                pc = pbig.tile([128, 512], f32, tag="pbig", name=f"pc{hb}")
                nc.tensor.matmul(pc[:], WSYM0, ohn_t, start=True, stop=True)
                cf = nsb.tile([128, 512], f16, tag="cf", name=f"cf{hb}")
                nc.any.tensor_copy(cf[:], pc[:])
                g = nsb.tile([128, 512], f16, tag="g", name=f"g{hb}")
                nc.vector.tensor_mul(g[:], pf[:], cf[:])
                po = pbig.tile([128, 512], f32, tag="pbig", name=f"po{hb}")
                nc.tensor.matmul(po[:], WLIN20, g[:], start=True, stop=False,
                                 skip_group_check=True)
                nc.tensor.matmul(po[:], SCT, ohn_t, start=False, stop=True,
                                 skip_group_check=True)
                ob = nsb.tile([128, 512], f16, tag="ob", name=f"ob{hb}")
                nc.any.tensor_copy(ob[:], po[:])
                pe_ = pmlp.tile([128, 512], f32, tag="pe", bufs=1,
                                name=f"pe{hb}")
                nc.tensor.matmul(pe_[0:1, :], WRO, ob[:], start=True,
                                 stop=False, skip_group_check=True)
                nc.tensor.matmul(pe_[0:1, :], AE, ohn_t, start=False,
                                 stop=True, skip_group_check=True)
                nc.any.tensor_copy(OUT[:, sl0:sl1], pe_[0:1, :])

            msgps = {}
            NHB = NPAD // 512
            for pi in range(NTT // 2):
                ti0 = 2 * pi
                if ti0 * 4 >= real_NS:
                    continue   # fully dead tile pair
                # transpose 2 tiles' ef -> one [128,256] buffer
                pefT = pmlp.tile([128, 256], f16, tag="pmlp")
                for k in range(2):
                    ti = ti0 + k
                    nc.tensor.transpose(
                        pefT[0:32, 128 * k:128 * k + 128],
                        EFALL[:, ti * 32:ti * 32 + 32], I128)
                efT = efTs[pi % 3]
                nc.scalar.copy(efT[0:32, :], pefT[0:32, :])

                # radial MLP for both tiles; block layout
                # [t0A, t1A, t0B, t1B] (A = subs 0,1 / B = subs 2,3)
                p1 = pmlp.tile([128, 512], f32, tag="pmlp")
                nc.tensor.matmul(p1[:, 0:256], W1A, efT[:],
                                 start=True, stop=True)
                nc.tensor.matmul(p1[:, 256:512], W1B, efT[:],
                                 start=True, stop=True)
                t1 = tp1.tile([128, 512], f16, tag="t1")
                nc.scalar.activation(t1[:], p1[:], AF.Silu)
                p2 = pmlp.tile([128, 512], f32, tag="pmlp")
                nc.tensor.matmul(p2[:], W2, t1[:], start=True, stop=True)
                t2 = tp2.tile([128, 512], f16, tag="t2")
                nc.scalar.activation(t2[:], p2[:], AF.Silu)
                p3 = pmlp.tile([128, 512], f32, tag="pmlp")
                nc.tensor.matmul(p3[:], W3, t2[:], start=True, stop=True)
                t3 = tp3.tile([128, 512], f16, tag="t3")
                nc.scalar.activation(t3[:], p3[:], AF.Silu)

                for k in range(2):
                    ti = ti0 + k
                    if ti * 4 >= real_NS:
                        continue   # fully dead tile
                    ht = htp.tile([128, 512], f16, tag="ht")
                    nc.sync.dma_start(ht[:], ht_d[ti])
                    # W4 with swapped operands: out = t3_block^T @ W4EO
                    # lands [edges, K] directly; W4EO rows 0:64 ->
                    # even-sub cols, 64:128 -> odd-sub
                    pRT = pbig.tile([128, 512], f32, tag="pbig")
                    nc.tensor.matmul(
                        pRT[:, 0:256], t3[:, 128 * k:128 * k + 128],
                        W4EO, start=True, stop=True)
                    nc.tensor.matmul(
                        pRT[:, 256:512], t3[:, 256 + 128 * k:384 + 128 * k],
                        W4EO, start=True, stop=True)
                    sA = sap.tile([128, 512], f8e4, tag="sA")
                    nc.vector.tensor_mul(sA[:], pRT[:], ht[:])
                    rlsl = RL[:, ti * 4:ti * 4 + 4]
                    ohr = ohrp.tile([128, 512], f8e5, tag="ohr")
                    nc.vector.tensor_tensor(
                        ohr[:].rearrange("p (s j) -> p s j", j=128),
                        IOTA.unsqueeze(1).broadcast_to([128, 4, 128]),
                        rlsl.unsqueeze(2).broadcast_to([128, 4, 128]),
                        OP.is_equal)
                    s = 0
                    while s < 4:
                        S = ti * 4 + s
                        if S >= real_NS:
                            break   # dead tail
                        n_ = sched[S]
                        if n_ not in msgps:
                            # <=2 node tiles open at once -> 2-deep rotation
                            msgps[n_] = pmsg.tile(
                                [128, 128], f32, tag="msgp",
                                name=f"msgp{n_}")
                        pair_ok = (s % 2 == 0 and S + 1 < real_NS
                                   and sched[S + 1] == n_)
                        if pair_ok:
                            # fp8 DoubleRow: contract both subtiles (256
                            # edges) in one instruction
                            nc.tensor.matmul(
                                msgps[n_][:],
                                sA[:, 128 * s:128 * s + 256].rearrange(
                                    "p (t k) -> p t k", t=2),
                                ohr[:, 128 * s:128 * s + 256].rearrange(
                                    "p (t k) -> p t k", t=2),
                                start=(S == first_S[n_]),
                                stop=(S + 1 == last_S[n_]),
                                perf_mode=DR,
                                skip_group_check=True)
                            s += 2
                        else:
                            nc.tensor.matmul(
                                msgps[n_][:],
                                sA[:, 128 * s:128 * s + 128],
                                ohr[:, 128 * s:128 * s + 128],
                                start=(S == first_S[n_]),
                                stop=(S == last_S[n_]),
                                skip_group_check=True)
                            s += 1
                    for s in range(4):
                        S = ti * 4 + s
                        if S >= real_NS:
                            continue
                        n_ = sched[S]
                        if S == last_S[n_]:
                            # node-tile messages to the epilogue buffer
                            nc.vector.tensor_scalar(
                                MS[:, 128 * n_:128 * n_ + 128],
                                msgps[n_][:], 1.0 / AVG, None, OP.mult)
                            # epilogue block as soon as its 4 nts land
                            if n_ % (NT // NHB) == NT // NHB - 1:
                                epilogue(n_ // (NT // NHB))

            nc.sync.dma_start(out_d[:], OUT[:])

    nc.compile()
    return nc


def _pack_w1(W1):
    """Pair p lhsT: rows 16p+{0:8} -> W1 cols 0:64, rows 16p+{8:16} ->
    W1 cols 64:128, zero elsewhere (full-128 contraction)."""
    out = []
    for p in range(2):
        q = np.zeros((128, 128), np.float32)
        q[16 * p + 0:16 * p + 8, 0:64] = W1
        q[16 * p + 8:16 * p + 16, 64:128] = W1
        out.append(q)
    return out


def _wbd(W):
    q = np.zeros((128, 128), np.float32)
    q[0:64, 0:64] = W
    q[64:128, 64:128] = W
    return q


def _pack_w4eo(W40):
    """[128, 256]: rows 0:64 -> cols 0:128 (even sub), rows 64:128 ->
    cols 128:256 (odd sub)."""
    q = np.zeros((128, 256), np.float32)
    q[0:64, 0:128] = W40
    q[64:128, 128:256] = W40
    return q


def _host_prep(inputs):
    """Reduce weights to tables and build per-core edge streams."""
    pos = np.asarray(inputs["positions"], np.float32)
    shifts = np.asarray(inputs["shifts"], np.float32)
    ei = np.asarray(inputs["edge_index"])
    species = np.asarray(inputs["species"]).astype(np.int64)
    ae = np.asarray(inputs["atomic_energies"], np.float32)
    w_embed = np.asarray(inputs["w_embed"], np.float32)
    w_up = np.asarray(inputs["w_up"], np.float32)
    W1 = np.asarray(inputs["W1"], np.float32)
    W2 = np.asarray(inputs["W2"], np.float32)
    W3 = np.asarray(inputs["W3"], np.float32)
    W4 = np.asarray(inputs["W4"], np.float32)
    w_lin = np.asarray(inputs["w_lin"], np.float32)
    w_skip = np.asarray(inputs["w_skip"], np.float32)
    w_sym = np.asarray(inputs["w_sym"], np.float32)
    w_lin2 = np.asarray(inputs["w_lin2"], np.float32)
    w_readout = np.asarray(inputs["w_readout"], np.float32)

    send, recv = ei[0].astype(np.int64), ei[1].astype(np.int64)
    vec = pos[recv] - pos[send] + shifts
    rsq = (vec * vec).sum(-1)
    keep = rsq < (R_MAX * R_MAX + 1e-3)   # envelope zero beyond cutoff
    vec = vec[keep]
    sp_s = species[send[keep]]
    recv = recv[keep]

    core = recv // NPC
    loc = recv % NPC
    ntile = loc // 128
    rl = (loc % 128).astype(np.float32)

    # group edges by (core, node tile)
    order = np.lexsort((ntile, core))
    vec, sp_s, rl = vec[order], sp_s[order], rl[order]
    core, ntile = core[order], ntile[order]
    gid = core * NT + ntile
    counts = np.bincount(gid, minlength=NCORES * NT).reshape(NCORES, NT)
    # shared cross-core slot layout: each node tile padded to the max
    # core's count rounded up to 128 (subtile granularity)
    padded = ((counts.max(axis=0) + 127) // 128 * 128).astype(np.int64)
    NTT = int(-(-padded.sum() // 1024)) * 2   # even: device pair loop
    offs = np.zeros(NT + 1, np.int64)
    np.cumsum(padded, out=offs[1:])

    # host-computed bessel*envelope edge features (smaller than raw vec)
    rr = np.sqrt((vec * vec).sum(-1) + EPS).astype(np.float32)
    nb = np.arange(1, NB + 1, dtype=np.float32)
    bess = (np.sqrt(2.0 / R_MAX) *
            np.sin(nb * np.pi * rr[:, None] / R_MAX) / rr[:, None])
    xx = rr / R_MAX
    P = 5.0
    env = (1.0 - 0.5 * (P + 1) * (P + 2) * xx ** 5 + P * (P + 2) * xx ** 6
           - 0.5 * P * (P + 1) * xx ** 7) * (xx < 1.0)
    ef8 = (bess * env[:, None]).astype(np.float16)   # [E, 8]

    EF = np.zeros((NCORES, NTT, 128, 4, 8), np.float16)
    RLH = np.zeros((NCORES, NTT, 128, 4), np.float16)
    # per-edge sender table h_up[species] in transposed layout [e, k];
    # pad slots stay zero so they scatter nothing
    HU16 = (w_embed @ w_up).astype(np.float16)
    HT4 = np.zeros((NCORES, NTT, 128, 4, 128), np.float16)

    starts = np.zeros(NCORES * NT + 1, np.int64)
    np.cumsum(counts.reshape(-1), out=starts[1:])
    for c_ in range(NCORES):
        for t in range(NT):
            g = c_ * NT + t
            a, b = starts[g], starts[g + 1]
            n = b - a
            s_, r_ = sp_s[a:b], rl[a:b]
            idx = np.arange(n) + offs[t]   # global slot within core
            ti = idx // 512
            sub = (idx % 512) // 128
            row = idx % 128
            EF[c_, ti, row, sub] = ef8[a:b]
            RLH[c_, ti, row, sub] = r_
            HT4[c_, ti, row, sub] = HU16[s_]

    OHN = np.zeros((NCORES, 10, NPAD), np.float16)
    for c_ in range(NCORES):
        sp_c = species[c_ * NPC:(c_ + 1) * NPC]
        OHN[c_, sp_c, np.arange(NPC)] = 1.0

    w1a, w1b = _pack_w1(W1)
    w4eo = _pack_w4eo(np.ascontiguousarray(W4.reshape(64, K, 4)[:, :, 0]))
    math = {
        "i128": np.eye(128, dtype=np.float32),
        "iota": np.broadcast_to(
            np.arange(128, dtype=np.float32), (128, 128)).copy(),
        "w1a": w1a, "w1b": w1b,
        "w2": _wbd(W2), "w3": _wbd(W3),
        "w4eo": w4eo,
        "wlin0": np.ascontiguousarray(w_lin[0]),
        "wsym0": np.ascontiguousarray(w_sym[0]),
        "wlin20": np.ascontiguousarray(w_lin2[0]),
        "sct": np.ascontiguousarray(
            np.einsum("zk,zkj->zj", w_embed, w_skip) / np.sqrt(Z)),
        "ae": ae.reshape(10, 1).copy(),
        "wro": w_readout.reshape(128, 1).copy(),
    }
    colsh = _spec_cols(_CONSTH_SPECS)
    packedh = np.zeros((128, CONSTH_W), np.float16)
    for k_, m in math.items():
        a, b = colsh[k_]
        packedh[:m.shape[0], a:b] = m.astype(np.float16)

    # device layouts (partition-major): EF [128, NTT*32], RL [128, NTT*4];
    # HT = [NTT, 128, 512] per core (one DMA per edge tile)
    EFD = EF.reshape(NCORES, NTT, 128, 32).transpose(0, 2, 1, 3).reshape(
        NCORES, 128, NTT * 32)
    RLD = RLH.transpose(0, 2, 1, 3).reshape(NCORES, 128, NTT * 4)
    HT = HT4.reshape(NCORES, NTT, 128, 512)
    key = tuple(int(p) for p in padded)
    return key, EFD, RLD, HT, OHN, {"consth": packedh}


def kernel(**inputs):
    global LAST_RESULTS
    from concourse.bass_utils import run_bass_kernel_spmd

    key, EFD, RLD, HT, OHN, consts = _host_prep(inputs)
    if key not in _prog_cache:
        _prog_cache[key] = _build_program(key)
    nc = _prog_cache[key]

    in_maps = []
    for c_ in range(NCORES):
        m = dict(consts)
        m["ef"] = np.ascontiguousarray(EFD[c_])
        m["rl"] = np.ascontiguousarray(RLD[c_])
        m["ht"] = np.ascontiguousarray(HT[c_])
        m["ohn"] = np.ascontiguousarray(OHN[c_])
        in_maps.append(m)

    res = run_bass_kernel_spmd(
        nc, in_maps, core_ids=list(range(NCORES)), trace=TRACE)
    LAST_RESULTS = res

    out = np.concatenate(
        [res.results[c_]["out"][0, :NPC] for c_ in range(NCORES)])
    return out.astype(np.float32)



# revision 95
# speedup vs baseline: 1.1735x; 1.1735x over previous
"""MACE-style GNN message passing on 8 Trainium2 NeuronCores.

Only the l=0 (scalar) channel of the reference network reaches the output:
Y[:,0] == 1 and the readout consumes out[:, :, 0] alone, so the whole l>=1
spherical-harmonic pipeline is dead code.  What remains per edge is the
radial MLP (8->64->64->64->128), a per-sender-species channel scale, and a
scatter-add over receivers.  Node-side work collapses to per-species table
lookups (implemented as one-hot matmuls) plus three K x K matmuls.

Sharding: receivers are range-partitioned over the 8 cores (1000 nodes
each, padded to 8 tiles of 128).  Each core gets the edges targeting its
nodes, grouped by 128-node tile and padded to a uniform segment length so
all cores run one SPMD program.  Edges with r >= r_max (zero envelope) are
dropped on the host.

All matmuls keep operands on full 128-partition tiles at base partition 0
(tile_position (0,0)); sub-128 contractions are realized with zero-padded
block weights.  Partial-partition operands on compute-written tiles
misread on this hardware.
"""

import sys
import numpy as np

sys.path.insert(0, "/opt/trn_rl_repo")

R_MAX = 5.0
EPS = 1e-9
AVG = 16.0
N_NODES = 8000
Z = 10
K = 128
NB = 8
NCORES = 8
NPC = N_NODES // NCORES       # nodes per core
NT = 8                        # node tiles per core (128 nodes each)
NPAD = NT * 128               # padded nodes per core
ET_EDGES = 512                # edges per device tile (4 x 128 subtiles)

# fp16 constants (matmul weights; PE runs fp16 at 1 cycle/row vs 4 for fp32)
_CONSTH_SPECS = [
    ("i128", 128), ("iota", 128),
    ("w1a", 128), ("w1b", 128),
    ("w2", 128), ("w3", 128),
    ("w4eo", 256),
    ("wlin0", 128), ("wsym0", 128),
    ("wlin20", 128), ("sct", 128), ("ae", 1), ("wro", 1),
]
CONSTH_W = sum(w for _, w in _CONSTH_SPECS)


def _spec_cols(specs):
    cols, off = {}, 0
    for name, w in specs:
        cols[name] = (off, off + w)
        off += w
    return cols


TRACE = False
LAST_RESULTS = None

_prog_cache = {}


def _build_program(padded):
    """Build the SPMD Bass program.  `padded` = slots per node tile
    (multiples of 128, shared across cores); one 512-edge device tile
    may span node tiles — scatter targets are baked per subtile."""
    from concourse import bass, bacc, mybir
    from concourse.tile import TileContext

    f32 = mybir.dt.float32
    f16 = mybir.dt.float16
    f8e4 = mybir.dt.float8e4
    f8e5 = mybir.dt.float8e5
    i32 = mybir.dt.int32
    AF = mybir.ActivationFunctionType
    OP = mybir.AluOpType
    DR = mybir.MatmulPerfMode.DoubleRow

    nc = bacc.Bacc(None, target_bir_lowering=False)
    tot = sum(padded)
    NTT = -(-tot // 1024) * 2   # device tiles per core (even: pair loop)
    NS = NTT * 4           # 128-edge subtiles
    real_NS = tot // 128   # live subtiles; dead tail skipped in scatter
    # subtile -> node tile schedule
    sched = []
    for t, p in enumerate(padded):
        sched += [t] * (p // 128)
    sched += [NT - 1] * (NS - len(sched))
    first_S = {}
    last_S = {}
    for S in range(real_NS):
        first_S.setdefault(sched[S], S)
        last_S[sched[S]] = S
    NE4 = NTT * 4          # (row, tile, subtile) scalar lanes
    NE32 = NTT * 32        # bessel feature lanes

    ef_d = nc.dram_tensor("ef", [128, NE32], f16, kind="ExternalInput")
    rl_d = nc.dram_tensor("rl", [128, NE4], f16, kind="ExternalInput")
    ht_d = nc.dram_tensor("ht", [NTT, 128, 512], f16, kind="ExternalInput")
    ohn_d = nc.dram_tensor("ohn", [10, NPAD], f16, kind="ExternalInput")
    consth_d = nc.dram_tensor("consth", [128, CONSTH_W], f16, kind="ExternalInput")
    out_d = nc.dram_tensor("out", [1, NPAD], f32, kind="ExternalOutput")

    PSUM = bass.MemorySpace.PSUM

    from contextlib import ExitStack

    with TileContext(nc) as tc:
        with ExitStack() as stack:
            cp = stack.enter_context(tc.tile_pool(name="const", bufs=1))
            htp = stack.enter_context(tc.tile_pool(name="htp", bufs=3))
            tp1 = stack.enter_context(tc.tile_pool(name="tp1", bufs=4))
            tp2 = stack.enter_context(tc.tile_pool(name="tp2", bufs=4))
            tp3 = stack.enter_context(tc.tile_pool(name="tp3", bufs=4))
            sap = stack.enter_context(tc.tile_pool(name="sap", bufs=3))
            ohrp = stack.enter_context(tc.tile_pool(name="ohrp", bufs=3))
            nsb = stack.enter_context(tc.tile_pool(name="nsb", bufs=2))
            pmlp = stack.enter_context(tc.tile_pool(name="pmlp", bufs=3, space=PSUM))
            pbig = stack.enter_context(tc.tile_pool(name="pbig", bufs=2, space=PSUM))
            pmsg = stack.enter_context(tc.tile_pool(name="pmsg", bufs=2, space=PSUM))

            # ---- inputs: weights + edge features first (critical path)
            CTH = cp.tile([128, CONSTH_W], f16)
            nc.sync.dma_start(CTH[:], consth_d[:])
            EFALL = cp.tile([128, NE32], f16)
            nc.sync.dma_start(EFALL[:], ef_d[:])
            RL = cp.tile([128, NE4], f16)
            nc.sync.dma_start(RL[:], rl_d[:])
            OHN = cp.tile([128, NPAD], f16)
            nc.sync.dma_start(OHN[0:10, :], ohn_d[:])
            ch = _spec_cols(_CONSTH_SPECS)
            IOTA = CTH[:, ch["iota"][0]:ch["iota"][1]]
            I128 = CTH[:, ch["i128"][0]:ch["i128"][1]]
            W1A = CTH[:, ch["w1a"][0]:ch["w1a"][1]]
            W1B = CTH[:, ch["w1b"][0]:ch["w1b"][1]]
            W2 = CTH[:, ch["w2"][0]:ch["w2"][1]]
            W3 = CTH[:, ch["w3"][0]:ch["w3"][1]]
            W4EO = CTH[:, ch["w4eo"][0]:ch["w4eo"][1]]
            WLIN0 = CTH[:, ch["wlin0"][0]:ch["wlin0"][1]]
            WSYM0 = CTH[0:10, ch["wsym0"][0]:ch["wsym0"][1]]
            WLIN20 = CTH[:, ch["wlin20"][0]:ch["wlin20"][1]]
            SCT = CTH[0:10, ch["sct"][0]:ch["sct"][1]]
            AE = CTH[0:10, ch["ae"][0]:ch["ae"][1]]
            WRO = CTH[:, ch["wro"][0]:ch["wro"][1]]
            OUT = cp.tile([1, NPAD], f32)
            MS = cp.tile([128, NPAD], f16)   # all node-tile messages

            # rotating efT buffers (2 tiles each); rows 32:128 stay zero
            efTs = [cp.tile([128, 256], f16, name=f"efT{i}") for i in range(3)]
            for b_ in efTs:
                nc.gpsimd.memset(b_[:], 0.0)

            def epilogue(hb):
                # per-512-node block: mix messages, element coef, readout
                sl0, sl1 = hb * 512, hb * 512 + 512
                pf = pbig.tile([128, 512], f32, tag="pbig", name=f"pf{hb}")
                nc.tensor.matmul(pf[:], WLIN0, MS[:, sl0:sl1],
                                 start=True, stop=True)
                ohn_t = OHN[0:10, sl0:sl1]
                pc = pbig.tile([128, 512], f32, tag="pbig", name=f"pc{hb}")
                nc.tensor.matmul(pc[:], WSYM0, ohn_t, start=True, stop=True)
                cf = nsb.tile([128, 512], f16, tag="cf", name=f"cf{hb}")
                nc.any.tensor_copy(cf[:], pc[:])
                g = nsb.tile([128, 512], f16, tag="g", name=f"g{hb}")
                nc.vector.tensor_mul(g[:], pf[:], cf[:])
                po = pbig.tile([128, 512], f32, tag="pbig", name=f"po{hb}")
                nc.tensor.matmul(po[:], WLIN20, g[:], start=True, stop=False,
                                 skip_group_check=True)
                nc.tensor.matmul(po[:], SCT, ohn_t, start=False, stop=True,
                                 skip_group_check=True)
                ob = nsb.tile([128, 512], f16, tag="ob", name=f"ob{hb}")
                nc.any.tensor_copy(ob[:], po[:])
                pe_ = pmlp.tile([128, 512], f32, tag="pe", bufs=1,
                                name=f"pe{hb}")
                nc.tensor.matmul(pe_[0:1, :], WRO, ob[:], start=True,
                                 stop=False, skip_group_check=True)
                nc.tensor.matmul(pe_[0:1, :], AE, ohn_t, start=False,
                                 stop=True, skip_group_check=True)
                nc.any.tensor_copy(OUT[:, sl0:sl1], pe_[0:1, :])

            msgps = {}
            NHB = NPAD // 512
            for pi in range(NTT // 2):
                ti0 = 2 * pi
                if ti0 * 4 >= real_NS:
                    continue   # fully dead tile pair
                # transpose 2 tiles' ef -> one [128,256] buffer
                pefT = pmlp.tile([128, 256], f16, tag="pmlp")
                for k in range(2):
                    ti = ti0 + k
                    nc.tensor.transpose(
                        pefT[0:32, 128 * k:128 * k + 128],
                        EFALL[:, ti * 32:ti * 32 + 32], I128)
                efT = efTs[pi % 3]
                nc.scalar.copy(efT[0:32, :], pefT[0:32, :])

                # radial MLP for both tiles; block layout
                # [t0A, t1A, t0B, t1B] (A = subs 0,1 / B = subs 2,3)
                p1 = pmlp.tile([128, 512], f32, tag="pmlp")
                nc.tensor.matmul(p1[:, 0:256], W1A, efT[:],
                                 start=True, stop=True)
                nc.tensor.matmul(p1[:, 256:512], W1B, efT[:],
                                 start=True, stop=True)
                t1 = tp1.tile([128, 512], f16, tag="t1")
                nc.scalar.activation(t1[:], p1[:], AF.Silu)
                p2 = pmlp.tile([128, 512], f32, tag="pmlp")
                nc.tensor.matmul(p2[:], W2, t1[:], start=True, stop=True)
                t2 = tp2.tile([128, 512], f16, tag="t2")
                nc.scalar.activation(t2[:], p2[:], AF.Silu)
                p3 = pmlp.tile([128, 512], f32, tag="pmlp")
                nc.tensor.matmul(p3[:], W3, t2[:], start=True, stop=True)
                t3 = tp3.tile([128, 512], f16, tag="t3")
                nc.scalar.activation(t3[:], p3[:], AF.Silu)

                for k in range(2):
                    ti = ti0 + k
                    if ti * 4 >= real_NS:
                        continue   # fully dead tile
                    ht = htp.tile([128, 512], f16, tag="ht")
                    nc.sync.dma_start(ht[:], ht_d[ti])
                    # W4 with swapped operands: out = t3_block^T @ W4EO
                    # lands [edges, K] directly; W4EO rows 0:64 ->
                    # even-sub cols, 64:128 -> odd-sub
                    pRT = pbig.tile([128, 512], f32, tag="pbig")
                    nc.tensor.matmul(
                        pRT[:, 0:256], t3[:, 128 * k:128 * k + 128],
                        W4EO, start=True, stop=True)
                    nc.tensor.matmul(
                        pRT[:, 256:512], t3[:, 256 + 128 * k:384 + 128 * k],
                        W4EO, start=True, stop=True)
                    sA = sap.tile([128, 512], f16, tag="sA")
                    nc.vector.tensor_mul(sA[:], pRT[:], ht[:])
                    rlsl = RL[:, ti * 4:ti * 4 + 4]
                    ohr = ohrp.tile([128, 512], f16, tag="ohr")
                    nc.vector.tensor_tensor(
                        ohr[:].rearrange("p (s j) -> p s j", j=128),
                        IOTA.unsqueeze(1).broadcast_to([128, 4, 128]),
                        rlsl.unsqueeze(2).broadcast_to([128, 4, 128]),
                        OP.is_equal)
                    for s in range(4):
                        S = ti * 4 + s
                        if S >= real_NS:
                            break   # dead tail
                        n_ = sched[S]
                        if n_ not in msgps:
                            # <=2 node tiles open at once -> 2-deep rotation
                            msgps[n_] = pmsg.tile(
                                [128, 128], f32, tag="msgp",
                                name=f"msgp{n_}")
                        nc.tensor.matmul(
                            msgps[n_][:],
                            sA[:, 128 * s:128 * s + 128],
                            ohr[:, 128 * s:128 * s + 128],
                            start=(S == first_S[n_]),
                            stop=(S == last_S[n_]),
                            skip_group_check=True)
                    for s in range(4):
                        S = ti * 4 + s
                        if S >= real_NS:
                            continue
                        n_ = sched[S]
                        if S == last_S[n_]:
                            # node-tile messages to the epilogue buffer
                            nc.vector.tensor_scalar(
                                MS[:, 128 * n_:128 * n_ + 128],
                                msgps[n_][:], 1.0 / AVG, None, OP.mult)
                            # epilogue block as soon as its 4 nts land
                            if n_ % (NT // NHB) == NT // NHB - 1:
                                epilogue(n_ // (NT // NHB))

            nc.sync.dma_start(out_d[:], OUT[:])

    nc.compile()
    return nc


def _pack_w1(W1):
    """Pair p lhsT: rows 16p+{0:8} -> W1 cols 0:64, rows 16p+{8:16} ->
    W1 cols 64:128, zero elsewhere (full-128 contraction)."""
    out = []
    for p in range(2):
        q = np.zeros((128, 128), np.float32)
        q[16 * p + 0:16 * p + 8, 0:64] = W1
        q[16 * p + 8:16 * p + 16, 64:128] = W1
        out.append(q)
    return out


def _wbd(W):
    q = np.zeros((128, 128), np.float32)
    q[0:64, 0:64] = W
    q[64:128, 64:128] = W
    return q


def _pack_w4eo(W40):
    """[128, 256]: rows 0:64 -> cols 0:128 (even sub), rows 64:128 ->
    cols 128:256 (odd sub)."""
    q = np.zeros((128, 256), np.float32)
    q[0:64, 0:128] = W40
    q[64:128, 128:256] = W40
    return q


def _host_prep(inputs):
    """Reduce weights to tables and build per-core edge streams."""
    pos = np.asarray(inputs["positions"], np.float32)
    shifts = np.asarray(inputs["shifts"], np.float32)
    ei = np.asarray(inputs["edge_index"])
    species = np.asarray(inputs["species"]).astype(np.int64)
    ae = np.asarray(inputs["atomic_energies"], np.float32)
    w_embed = np.asarray(inputs["w_embed"], np.float32)
    w_up = np.asarray(inputs["w_up"], np.float32)
    W1 = np.asarray(inputs["W1"], np.float32)
    W2 = np.asarray(inputs["W2"], np.float32)
    W3 = np.asarray(inputs["W3"], np.float32)
    W4 = np.asarray(inputs["W4"], np.float32)
    w_lin = np.asarray(inputs["w_lin"], np.float32)
    w_skip = np.asarray(inputs["w_skip"], np.float32)
    w_sym = np.asarray(inputs["w_sym"], np.float32)
    w_lin2 = np.asarray(inputs["w_lin2"], np.float32)
    w_readout = np.asarray(inputs["w_readout"], np.float32)

    send, recv = ei[0].astype(np.int64), ei[1].astype(np.int64)
    vec = pos[recv] - pos[send] + shifts
    rsq = (vec * vec).sum(-1)
    keep = rsq < (R_MAX * R_MAX + 1e-3)   # envelope zero beyond cutoff
    vec = vec[keep]
    sp_s = species[send[keep]]
    recv = recv[keep]

    core = recv // NPC
    loc = recv % NPC
    ntile = loc // 128
    rl = (loc % 128).astype(np.float32)

    # group edges by (core, node tile)
    order = np.lexsort((ntile, core))
    vec, sp_s, rl = vec[order], sp_s[order], rl[order]
    core, ntile = core[order], ntile[order]
    gid = core * NT + ntile
    counts = np.bincount(gid, minlength=NCORES * NT).reshape(NCORES, NT)
    # shared cross-core slot layout: each node tile padded to the max
    # core's count rounded up to 128 (subtile granularity)
    padded = ((counts.max(axis=0) + 127) // 128 * 128).astype(np.int64)
    NTT = int(-(-padded.sum() // 1024)) * 2   # even: device pair loop
    offs = np.zeros(NT + 1, np.int64)
    np.cumsum(padded, out=offs[1:])

    # host-computed bessel*envelope edge features (smaller than raw vec)
    rr = np.sqrt((vec * vec).sum(-1) + EPS).astype(np.float32)
    nb = np.arange(1, NB + 1, dtype=np.float32)
    bess = (np.sqrt(2.0 / R_MAX) *
            np.sin(nb * np.pi * rr[:, None] / R_MAX) / rr[:, None])
    xx = rr / R_MAX
    P = 5.0
    env = (1.0 - 0.5 * (P + 1) * (P + 2) * xx ** 5 + P * (P + 2) * xx ** 6
           - 0.5 * P * (P + 1) * xx ** 7) * (xx < 1.0)
    ef8 = (bess * env[:, None]).astype(np.float16)   # [E, 8]

    EF = np.zeros((NCORES, NTT, 128, 4, 8), np.float16)
    RLH = np.zeros((NCORES, NTT, 128, 4), np.float16)
    # per-edge sender table h_up[species] in transposed layout [e, k];
    # pad slots stay zero so they scatter nothing
    HU16 = (w_embed @ w_up).astype(np.float16)
    HT4 = np.zeros((NCORES, NTT, 128, 4, 128), np.float16)

    starts = np.zeros(NCORES * NT + 1, np.int64)
    np.cumsum(counts.reshape(-1), out=starts[1:])
    for c_ in range(NCORES):
        for t in range(NT):
            g = c_ * NT + t
            a, b = starts[g], starts[g + 1]
            n = b - a
            s_, r_ = sp_s[a:b], rl[a:b]
            idx = np.arange(n) + offs[t]   # global slot within core
            ti = idx // 512
            sub = (idx % 512) // 128
            row = idx % 128
            EF[c_, ti, row, sub] = ef8[a:b]
            RLH[c_, ti, row, sub] = r_
            HT4[c_, ti, row, sub] = HU16[s_]

    OHN = np.zeros((NCORES, 10, NPAD), np.float16)
    for c_ in range(NCORES):
        sp_c = species[c_ * NPC:(c_ + 1) * NPC]
        OHN[c_, sp_c, np.arange(NPC)] = 1.0

    w1a, w1b = _pack_w1(W1)
    w4eo = _pack_w4eo(np.ascontiguousarray(W4.reshape(64, K, 4)[:, :, 0]))
    math = {
        "i128": np.eye(128, dtype=np.float32),
        "iota": np.broadcast_to(
            np.arange(128, dtype=np.float32), (128, 128)).copy(),
        "w1a": w1a, "w1b": w1b,
        "w2": _wbd(W2), "w3": _wbd(W3),
        "w4eo": w4eo,
        "wlin0": np.ascontiguousarray(w_lin[0]),
        "wsym0": np.ascontiguousarray(w_sym[0]),
        "wlin20": np.ascontiguousarray(w_lin2[0]),
        "sct": np.ascontiguousarray(
            np.einsum("zk,zkj->zj", w_embed, w_skip) / np.sqrt(Z)),
        "ae": ae.reshape(10, 1).copy(),
        "wro": w_readout.reshape(128, 1).copy(),
    }
    colsh = _spec_cols(_CONSTH_SPECS)
    packedh = np.zeros((128, CONSTH_W), np.float16)
    for k_, m in math.items():
        a, b = colsh[k_]
        packedh[:m.shape[0], a:b] = m.astype(np.float16)

    # device layouts (partition-major): EF [128, NTT*32], RL [128, NTT*4];
    # HT = [NTT, 128, 512] per core (one DMA per edge tile)
    EFD = EF.reshape(NCORES, NTT, 128, 32).transpose(0, 2, 1, 3).reshape(
        NCORES, 128, NTT * 32)
    RLD = RLH.transpose(0, 2, 1, 3).reshape(NCORES, 128, NTT * 4)
    HT = HT4.reshape(NCORES, NTT, 128, 512)
    key = tuple(int(p) for p in padded)
    return key, EFD, RLD, HT, OHN, {"consth": packedh}


def kernel(**inputs):
    global LAST_RESULTS
    from concourse.bass_utils import run_bass_kernel_spmd

    key, EFD, RLD, HT, OHN, consts = _host_prep(inputs)
    if key not in _prog_cache:
        _prog_cache[key] = _build_program(key)
    nc = _prog_cache[key]

    in_maps = []
    for c_ in range(NCORES):
        m = dict(consts)
        m["ef"] = np.ascontiguousarray(EFD[c_])
        m["rl"] = np.ascontiguousarray(RLD[c_])
        m["ht"] = np.ascontiguousarray(HT[c_])
        m["ohn"] = np.ascontiguousarray(OHN[c_])
        in_maps.append(m)

    res = run_bass_kernel_spmd(
        nc, in_maps, core_ids=list(range(NCORES)), trace=TRACE)
    LAST_RESULTS = res

    out = np.concatenate(
        [res.results[c_]["out"][0, :NPC] for c_ in range(NCORES)])
    return out.astype(np.float32)

